# revision 23
# baseline (speedup 1.0000x reference)
"""Trainium2 Bass kernel for nn_Conv1d_NN (retrieval_knn).

Per batch: pairwise L2 distances over N=4096 positions (C=256 dims),
top-3 nearest indices per row (self + 2 NN), gather neighbor columns,
K=3 stride-3 Conv1d == sum_k W_k @ x[:, idx[:, k]] + b.

Sharding: data-parallel over batch B=16 across 8 cores (2 batches/core).

Design (v5):
- Distances: nd = x_i.x_j - sq_j/2 in PSUM via fp16 hh product (2 matmuls)
  + fp8 DoubleRow correction products h.l' + l.h' (e4m3 x e5m2, K=256 in
  one DR matmul each at 0.5 cy/row) + 2-row fp16 hi/lo bias fold + a
  -BIG identity matmul killing the diagonal (self index is known).
  Measured nd abs err ~2.6e-4 rms (fail line ~8e-3): 2x fewer PE cycles
  than the fp16 hi/lo 3-product scheme.
- Scan: ACT evicts each PSUM chunk to a double-buffered SBUF row copy
  (nds [128, 4096] fp32); DVE then runs ONE full-row max8 + ONE
  full-row max_index on it. Self is killed by the diag matmul, so
  ranks 0,1 of the top-8 are nn1, nn2 with exact indices -- no chunk
  merges, no index-recovery passes, no gathers. (Full-row scans on an
  SBUF copy beat chunked PSUM scans: no per-chunk candidate merge, and
  PSUM banks are released by the ACT copy, decoupling PE.)
- Conv: Y1^T/Y2^T = (x^T W_k^T) fp16 -> DRAM -> indirect row gather;
  Y0 computed directly in [o, n]; gathered [n, o] tiles PE-transpose
  (fp16 identity matmul) into Y0's PSUM; one ACT eviction adds bias.
- Batch emission interleaved as in v3 so prep/YkT of batch b+1 and conv
  of batch b fill the PE stream between the two phase-A sweeps.
"""

import contextlib
import sys

sys.path.insert(0, "/opt/trn_rl_repo")

import numpy as np

import concourse.bacc as bacc
import concourse.mybir as mybir
from concourse.bass import IndirectOffsetOnAxis
from concourse.bass_utils import run_bass_kernel_spmd
from concourse.masks import make_identity
from concourse.tile import TileContext

F32 = mybir.dt.float32
F16 = mybir.dt.float16
F8E4 = mybir.dt.float8e4
F8E5 = mybir.dt.float8e5
U32 = mybir.dt.uint32
AF = mybir.ActivationFunctionType
ALU = mybir.AluOpType
DR = mybir.MatmulPerfMode.DoubleRow
AXX = mybir.AxisListType.X

B, C, N, K = 16, 256, 4096, 3
NCORES = 8
BPC = B // NCORES  # batches per core
P = 128
NB = N // P        # 32 row-blocks of 128
CH = C // P        # 2 contraction halves
JT = 512           # matmul moving tile (one PSUM bank of fp32)
CHW = 1024         # phase-A PSUM chunk width (2 PSUM banks)
NCHK = N // CHW    # 4 chunks per row sweep
W8 = 8             # A-window size / B class count
NWIN = N // W8     # 512 A-windows
BIG = 30000.0
W12_OFF = CH * C   # wt_sb: [W0_h0 | W0_h1 | W1_h0 W2_h0 | W1_h1 W2_h1]


class BatchCtx:
    pass


def _prep_data(nc, tc, bc):
    """Load x, build fp16 hi + fp8 hi/lo (e4m3/e5m2)."""
    b = bc.b
    bc.xf = []
    for h in range(CH):
        xf = bc.scratch.tile([P, N], F32, tag=f"xf{h}", name=f"xf_{b}_{h}")
        nc.sync.dma_start(out=xf, in_=bc.x_in[b, h * P:(h + 1) * P, :])
        nc.scalar.activation(bc.xh16[h], xf, func=AF.Copy)
        nc.gpsimd.tensor_tensor(out=bc.l8[:, h, :], in0=xf,
                                in1=bc.xh16[h], op=ALU.subtract)
        nc.scalar.activation(bc.h8[:, h, :], bc.xh16[h], func=AF.Copy)
        bc.xf.append(xf)


def _prep_sq(nc, tc, bc):
    """sq ones-matmul + -sq/2 fp16 hi/lo pair rows."""
    b = bc.b
    with tc.tile_pool(name=f"sqc{b}", bufs=1) as sqc:
        for hj in range(N // JT):
            js = slice(hj * JT, (hj + 1) * JT)
            pq = bc.pmisc.tile([1, JT], F32, tag="pm", name=f"pq_{b}_{hj}")
            for h in range(CH):
                xxt = sqc.tile([P, JT], F32, tag="xxt",
                               name=f"xxt_{b}_{hj}_{h}")
                nc.vector.tensor_tensor(out=xxt, in0=bc.xf[h][:, js],
                                        in1=bc.xf[h][:, js], op=ALU.mult)
                nc.tensor.matmul(pq[0:1, :], bc.ones_col, xxt,
                                 start=(h == 0), stop=(h == CH - 1))
            sqf = sqc.tile([1, JT], F32, tag="sqf", name=f"sqf_{b}_{hj}")
            sql = sqc.tile([1, JT], F16, tag="sql", name=f"sql_{b}_{hj}")
            nc.scalar.activation(sqf[0:1, :], pq[0:1, :], func=AF.Copy,
                                 scale=-0.5)
            nc.scalar.activation(bc.nsq[0:1, js], sqf[0:1, :], func=AF.Copy)
            nc.gpsimd.tensor_tensor(out=sql[0:1, :], in0=sqf[0:1, :],
                                    in1=bc.nsq[0:1, js], op=ALU.subtract)
            nc.sync.dma_start(out=bc.nsq[1:2, js], in_=sql[0:1, :])


def _ykt(nc, tc, bc):
    """[Y1^T | Y2^T] = x^T [W1^T | W2^T] in fp16 -> DRAM for row gathers."""
    b = bc.b
    with tc.tile_pool(name=f"yk{b}", bufs=3) as ykp:
        for ib in range(NB):
            ibs = slice(ib * P, (ib + 1) * P)
            pk = bc.pmisc.tile([P, 2 * C], F32, tag="pm", name=f"yk_{b}_{ib}")
            for h in range(CH):
                ws = slice(W12_OFF + h * 2 * C, W12_OFF + (h + 1) * 2 * C)
                nc.tensor.matmul(pk, bc.xh16[h][:, ibs], bc.wt_sb[:, ws],
                                 start=(h == 0), stop=(h == CH - 1))
            ysb = ykp.tile([P, 2 * C], F16, tag="ysb", name=f"ysb_{b}_{ib}")
            nc.scalar.activation(ysb, pk, func=AF.Copy)
            for ki in range(2):
                nc.sync.dma_start(out=bc.ykt_d[ki][ibs, :],
                                  in_=ysb[:, ki * C:(ki + 1) * C])


def _phase_a_block(nc, tc, bc, psA, ib):
    """One row-block: distance chunks -> evict -> full-row top-8 scan ->
    j indices -> conv row gathers + interleaved conv output chunks."""
    b = bc.b
    mg = bc.mg
    if True:
        if True:
            ibs = slice(ib * P, (ib + 1) * P)
            nds = bc.big.tile([P, N], F32, tag="nds",
                              name=f"nds_{b}_{ib}")
            for g in range(2):
                # 4 one-bank PSUM chunks live at once; product-major
                # emission across them gives a depth-4 interleave so the
                # per-region accumulation chains pipeline on PE
                Q = 4
                pss = [psA.tile([P, JT], F32, tag="nd",
                                name=f"nd_{b}_{ib}_{g}_{q}")
                       for q in range(Q)]
                gj0s = [(g * Q + q) * JT for q in range(Q)]
                for h in range(CH):
                    for q in range(Q):
                        gjs = slice(gj0s[q], gj0s[q] + JT)
                        nc.tensor.matmul(pss[q], bc.xh16[h][:, ibs],
                                         bc.xh16[h][:, gjs],
                                         start=(h == 0), stop=False)
                for stat, mov in ((bc.h8, bc.l8), (bc.l8, bc.h8)):
                    for q in range(Q):
                        gjs = slice(gj0s[q], gj0s[q] + JT)
                        nc.tensor.matmul(pss[q], stat[:, :, ibs],
                                         mov[:, :, gjs], start=False,
                                         stop=False, perf_mode=DR)
                for q in range(Q):
                    gjs = slice(gj0s[q], gj0s[q] + JT)
                    has_diag = gj0s[q] <= ib * P < gj0s[q] + JT
                    nc.tensor.matmul(pss[q], bc.ones2, bc.nsq[:, gjs],
                                     start=False, stop=not has_diag)
                for q in range(Q):
                    if gj0s[q] <= ib * P < gj0s[q] + JT:
                        ds = slice(ib * P - gj0s[q], ib * P - gj0s[q] + P)
                        nc.tensor.matmul(pss[q][:, ds], bc.ident16,
                                         bc.identbig, start=False, stop=True)
                for q in range(Q):
                    cs = slice(gj0s[q], gj0s[q] + JT)
                    nc.scalar.activation(nds[:, cs], pss[q], func=AF.Copy)
            # full-row top-8 scan on the evicted SBUF copy; self killed,
            # so ranks 0,1 are nn1, nn2 with exact indices from max_index
            m8a = mg.tile([P, 8], F32, tag="m8a", name=f"m8a_{b}_{ib}")
            i8a = mg.tile([P, 8], U32, tag="i8a", name=f"i8a_{b}_{ib}")
            nc.vector.max(out=m8a, in_=nds)
            nc.vector.max_index(out=i8a, in_max=m8a, in_values=nds)
            nc.gpsimd.tensor_copy(bc.idx12u[:, 2 * ib:2 * ib + 2],
                                  i8a[:, 0:2])
            # conv row gathers + g1+g2 sum
            g1 = mg.tile([P, C], F16, tag="g1", name=f"g1_{b}_{ib}")
            g2 = mg.tile([P, C], F16, tag="g2", name=f"g2_{b}_{ib}")
            nc.gpsimd.indirect_dma_start(
                out=g1, out_offset=None, in_=bc.ykt_d[0][:, :],
                in_offset=IndirectOffsetOnAxis(
                    ap=bc.idx12u[:, 2 * ib:2 * ib + 1], axis=0))
            nc.gpsimd.indirect_dma_start(
                out=g2, out_offset=None, in_=bc.ykt_d[1][:, :],
                in_offset=IndirectOffsetOnAxis(
                    ap=bc.idx12u[:, 2 * ib + 1:2 * ib + 2], axis=0))
            nc.gpsimd.tensor_tensor(out=bc.g12[:, ib * C:(ib + 1) * C],
                                    in0=g1, in1=g2, op=ALU.add)
            # conv output chunks interleave into the phase-A PE stream
            if ib >= 5 and (ib - 5) % (JT // P) == 0:
                _conv_chunk(nc, tc, bc, (ib - 5) // (JT // P))
            if ib == NB - 1:
                _conv_chunk(nc, tc, bc, NB // (JT // P) - 1)


def _conv_chunk(nc, tc, bc, ncn):
    """Y0 + transposed gather accumulate + biased eviction + out DMA
    for one 512-wide output chunk."""
    b = bc.b
    nsl = slice(ncn * JT, (ncn + 1) * JT)
    for oh in range(CH):
        ohs = slice(oh * P, (oh + 1) * P)
        pso = bc.pmisc.tile([P, JT], F32, tag="pm",
                            name=f"pso_{b}_{ncn}_{oh}")
        for h in range(CH):
            ws = slice(h * C + oh * P, h * C + (oh + 1) * P)
            nc.tensor.matmul(pso, bc.wt_sb[:, ws], bc.xh16[h][:, nsl],
                             start=(h == 0), stop=False)
        for nb4 in range(JT // P):
            ib = ncn * (JT // P) + nb4
            bs = slice(nb4 * P, (nb4 + 1) * P)
            gsl = slice(ib * C + oh * P, ib * C + (oh + 1) * P)
            nc.tensor.matmul(pso[:, bs], bc.g12[:, gsl], bc.ident16,
                             start=False, stop=True)
        osb = bc.mg.tile([P, JT], F32, tag="osb", name=f"osb_{b}_{ncn}_{oh}")
        nc.scalar.activation(osb, pso, func=AF.Identity,
                             bias=bc.biasc[oh][:, 0:1])
        nc.sync.dma_start(out=bc.out_t[b, ohs, nsl], in_=osb)


def build():
    nc = bacc.Bacc(None, target_bir_lowering=False)
    x_in = nc.dram_tensor("x", [BPC, C, N], F32, kind="ExternalInput")
    wt_in = nc.dram_tensor("wt", [K, C, C], F16, kind="ExternalInput")
    bias_in = nc.dram_tensor("bias", [C, 1], F32, kind="ExternalInput")
    out_t = nc.dram_tensor("out", [BPC, C, N], F32, kind="ExternalOutput")

    with TileContext(nc) as tc, contextlib.ExitStack() as es:
        constp = es.enter_context(tc.tile_pool(name="const", bufs=1))
        ident16 = constp.tile([P, P], F16)
        identbig = constp.tile([P, P], F16)
        ones_col = constp.tile([P, 1], F32)
        ones2 = constp.tile([2, P], F16)
        wt_sb = constp.tile([P, K * CH * C], F16)
        biasc = [constp.tile([P, 1], F32, tag=f"bc{oh}", name=f"bc{oh}")
                 for oh in range(CH)]
        make_identity(nc, ident16)
        nc.scalar.activation(identbig, ident16, func=AF.Copy, scale=-BIG)
        nc.vector.memset(ones_col, 1.0)
        nc.vector.memset(ones2, 1.0)
        for oh in range(CH):
            nc.sync.dma_start(out=biasc[oh],
                              in_=bias_in[oh * P:(oh + 1) * P, :])
        for h in range(CH):
            nc.sync.dma_start(out=wt_sb[:, h * C:(h + 1) * C],
                              in_=wt_in[0, h * P:(h + 1) * P, :])
            for k in (1, 2):
                ws = slice(W12_OFF + h * 2 * C + (k - 1) * C,
                           W12_OFF + h * 2 * C + k * C)
                nc.sync.dma_start(out=wt_sb[:, ws],
                                  in_=wt_in[k, h * P:(h + 1) * P, :])

        xb = es.enter_context(tc.tile_pool(name="xb", bufs=1))
        big = es.enter_context(tc.tile_pool(name="big", bufs=2))
        mg = es.enter_context(tc.tile_pool(name="mg", bufs=3))
        scratch = es.enter_context(tc.tile_pool(name="scratch", bufs=1))
        pmisc = es.enter_context(
            tc.tile_pool(name="pmisc", bufs=4, space="PSUM"))
        ydr = es.enter_context(tc.tile_pool(name="ydr", bufs=1, space="DRAM"))
        bcs = []
        for b in range(BPC):
            bc = BatchCtx()
            bc.b, bc.mg, bc.scratch, bc.xx = b, mg, scratch, []
            bc.big = big
            bc.pmisc, bc.out_t = pmisc, out_t
            bc.x_in, bc.wt_sb, bc.biasc = x_in, wt_sb, biasc
            bc.ident16, bc.identbig = ident16, identbig
            bc.ones_col, bc.ones2 = ones_col, ones2
            bc.xh16 = [xb.tile([P, N], F16, tag=f"xh{h}_{b}",
                               name=f"xh_{b}_{h}") for h in range(CH)]
            bc.h8 = xb.tile([P, CH, N], F8E4, tag=f"h8_{b}", name=f"h8_{b}")
            bc.l8 = xb.tile([P, CH, N], F8E5, tag=f"l8_{b}", name=f"l8_{b}")
            bc.nsq = xb.tile([2, N], F16, tag=f"nsq_{b}", name=f"nsq_{b}")
            bc.idx12u = xb.tile([P, NB * 2], U32, tag=f"i12u_{b}",
                                name=f"i12u_{b}")
            bc.g12 = xb.tile([P, NB * C], F16, tag=f"g12_{b}",
                             name=f"g12_{b}")
            bc.ykt_d = [ydr.tile([N, C], F16, tag=f"y{k}t_{b}",
                                 name=f"y{k}t_{b}") for k in (1, 2)]
            bcs.append(bc)

        for b in range(BPC):
            _prep_data(nc, tc, bcs[b])
            _prep_sq(nc, tc, bcs[b])
        # dummy matmuls keep the PE HAM monitor busy (warm 2.4 GHz)
        warm = pmisc.tile([P, P], F32, tag="pm", name="warm")
        for _ in range(96):
            nc.tensor.matmul(warm, ident16, ident16, start=True, stop=True)
        for b in range(BPC):
            _ykt(nc, tc, bcs[b])
        with tc.tile_pool(name="psA", bufs=4, space="PSUM") as psA:
            for ib in range(NB):
                for b in range(BPC):
                    _phase_a_block(nc, tc, bcs[b], psA, ib)
    nc.compile()
    return nc


_NC = None


def _get_nc():
    global _NC
    if _NC is None:
        _NC = build()
    return _NC


def make_in_maps(x, W, b):
    x = np.ascontiguousarray(x, dtype=np.float32)
    wt = np.ascontiguousarray(np.transpose(W, (2, 1, 0))).astype(np.float16)
    bias = np.ascontiguousarray(b, dtype=np.float32).reshape(C, 1)
    return [
        {"x": np.ascontiguousarray(x[i * BPC:(i + 1) * BPC]),
         "wt": wt, "bias": bias}
        for i in range(NCORES)
    ]


def kernel(x, W, b):
    nc = _get_nc()
    in_maps = make_in_maps(x, W, b)
    res = run_bass_kernel_spmd(nc, in_maps, core_ids=list(range(NCORES))).results
    return np.concatenate([r["out"] for r in res], axis=0)


# revision 24
# speedup vs baseline: 1.0344x; 1.0344x over previous
"""Trainium2 Bass kernel for nn_Conv1d_NN (retrieval_knn).

Per batch: pairwise L2 distances over N=4096 positions (C=256 dims),
top-3 nearest indices per row (self + 2 NN), gather neighbor columns,
K=3 stride-3 Conv1d == sum_k W_k @ x[:, idx[:, k]] + b.

Sharding: data-parallel over batch B=16 across 8 cores (2 batches/core).

Design (v5):
- Distances: nd = x_i.x_j - sq_j/2 in PSUM via fp16 hh product (2 matmuls)
  + fp8 DoubleRow correction products h.l' + l.h' (e4m3 x e5m2, K=256 in
  one DR matmul each at 0.5 cy/row) + 2-row fp16 hi/lo bias fold + a
  -BIG identity matmul killing the diagonal (self index is known).
  Measured nd abs err ~2.6e-4 rms (fail line ~8e-3): 2x fewer PE cycles
  than the fp16 hi/lo 3-product scheme.
- Scan: ACT evicts each PSUM chunk to a double-buffered SBUF row copy
  (nds [128, 4096] fp32); DVE then runs ONE full-row max8 + ONE
  full-row max_index on it. Self is killed by the diag matmul, so
  ranks 0,1 of the top-8 are nn1, nn2 with exact indices -- no chunk
  merges, no index-recovery passes, no gathers. (Full-row scans on an
  SBUF copy beat chunked PSUM scans: no per-chunk candidate merge, and
  PSUM banks are released by the ACT copy, decoupling PE.)
- Conv: Y1^T/Y2^T = (x^T W_k^T) fp16 -> DRAM -> indirect row gather;
  Y0 computed directly in [o, n]; gathered [n, o] tiles PE-transpose
  (fp16 identity matmul) into Y0's PSUM; one ACT eviction adds bias.
- Batch emission interleaved as in v3 so prep/YkT of batch b+1 and conv
  of batch b fill the PE stream between the two phase-A sweeps.
"""

import contextlib
import sys

sys.path.insert(0, "/opt/trn_rl_repo")

import numpy as np

import concourse.bacc as bacc
import concourse.mybir as mybir
from concourse.bass import IndirectOffsetOnAxis
from concourse.bass_utils import run_bass_kernel_spmd
from concourse.masks import make_identity
from concourse.tile import TileContext

F32 = mybir.dt.float32
F16 = mybir.dt.float16
F8E4 = mybir.dt.float8e4
F8E5 = mybir.dt.float8e5
U32 = mybir.dt.uint32
AF = mybir.ActivationFunctionType
ALU = mybir.AluOpType
DR = mybir.MatmulPerfMode.DoubleRow
AXX = mybir.AxisListType.X

B, C, N, K = 16, 256, 4096, 3
NCORES = 8
BPC = B // NCORES  # batches per core
P = 128
NB = N // P        # 32 row-blocks of 128
CH = C // P        # 2 contraction halves
JT = 512           # matmul moving tile (one PSUM bank of fp32)
CHW = 1024         # phase-A PSUM chunk width (2 PSUM banks)
NCHK = N // CHW    # 4 chunks per row sweep
W8 = 8             # A-window size / B class count
NWIN = N // W8     # 512 A-windows
BIG = 30000.0
W12_OFF = CH * C   # wt_sb: [W0_h0 | W0_h1 | W1_h0 W2_h0 | W1_h1 W2_h1]


class BatchCtx:
    pass


def _prep_data(nc, tc, bc):
    """Load x, build fp16 hi + fp8 hi/lo (e4m3/e5m2)."""
    b = bc.b
    bc.xf = []
    for h in range(CH):
        xf = bc.scratch.tile([P, N], F32, tag=f"xf{h}", name=f"xf_{b}_{h}")
        nc.sync.dma_start(out=xf, in_=bc.x_in[b, h * P:(h + 1) * P, :])
        nc.scalar.activation(bc.xh16[h], xf, func=AF.Copy)
        nc.gpsimd.tensor_tensor(out=bc.l8[:, h, :], in0=xf,
                                in1=bc.xh16[h], op=ALU.subtract)
        nc.scalar.activation(bc.h8[:, h, :], bc.xh16[h], func=AF.Copy)
        bc.xf.append(xf)


def _prep_sq(nc, tc, bc):
    """sq ones-matmul + -sq/2 fp16 hi/lo pair rows."""
    b = bc.b
    with tc.tile_pool(name=f"sqc{b}", bufs=1) as sqc:
        for hj in range(N // JT):
            js = slice(hj * JT, (hj + 1) * JT)
            pq = bc.pmisc.tile([1, JT], F32, tag="pm", name=f"pq_{b}_{hj}")
            for h in range(CH):
                xxt = sqc.tile([P, JT], F32, tag="xxt",
                               name=f"xxt_{b}_{hj}_{h}")
                nc.vector.tensor_tensor(out=xxt, in0=bc.xf[h][:, js],
                                        in1=bc.xf[h][:, js], op=ALU.mult)
                nc.tensor.matmul(pq[0:1, :], bc.ones_col, xxt,
                                 start=(h == 0), stop=(h == CH - 1))
            sqf = sqc.tile([1, JT], F32, tag="sqf", name=f"sqf_{b}_{hj}")
            sql = sqc.tile([1, JT], F16, tag="sql", name=f"sql_{b}_{hj}")
            nc.scalar.activation(sqf[0:1, :], pq[0:1, :], func=AF.Copy,
                                 scale=-0.5)
            nc.scalar.activation(bc.nsq[0:1, js], sqf[0:1, :], func=AF.Copy)
            nc.gpsimd.tensor_tensor(out=sql[0:1, :], in0=sqf[0:1, :],
                                    in1=bc.nsq[0:1, js], op=ALU.subtract)
            nc.sync.dma_start(out=bc.nsq[1:2, js], in_=sql[0:1, :])


def _ykt(nc, tc, bc):
    """[Y1^T | Y2^T] = x^T [W1^T | W2^T] in fp16 -> DRAM for row gathers."""
    b = bc.b
    with tc.tile_pool(name=f"yk{b}", bufs=3) as ykp:
        for ib in range(NB):
            ibs = slice(ib * P, (ib + 1) * P)
            pk = bc.pmisc.tile([P, 2 * C], F32, tag="pm", name=f"yk_{b}_{ib}")
            for h in range(CH):
                ws = slice(W12_OFF + h * 2 * C, W12_OFF + (h + 1) * 2 * C)
                nc.tensor.matmul(pk, bc.xh16[h][:, ibs], bc.wt_sb[:, ws],
                                 start=(h == 0), stop=(h == CH - 1))
            ysb = ykp.tile([P, 2 * C], F16, tag="ysb", name=f"ysb_{b}_{ib}")
            nc.scalar.activation(ysb, pk, func=AF.Copy)
            for ki in range(2):
                nc.sync.dma_start(out=bc.ykt_d[ki][ibs, :],
                                  in_=ysb[:, ki * C:(ki + 1) * C])


def _phase_a(nc, tc, bc):
    """Distance chunks -> evict -> full-row top-8 scan -> j indices ->
    conv row gathers. Conv output chunks interleave into the PE stream."""
    b = bc.b
    mg = bc.mg
    with tc.tile_pool(name=f"psA{b}", bufs=4, space="PSUM") as psA:
        for ib in range(NB):
            ibs = slice(ib * P, (ib + 1) * P)
            nds = bc.big.tile([P, N], F32, tag="nds",
                              name=f"nds_{b}_{ib}")
            for g in range(2):
                # 4 one-bank PSUM chunks live at once; product-major
                # emission across them gives a depth-4 interleave so the
                # per-region accumulation chains pipeline on PE
                Q = 4
                pss = [psA.tile([P, JT], F32, tag="nd",
                                name=f"nd_{b}_{ib}_{g}_{q}")
                       for q in range(Q)]
                gj0s = [(g * Q + q) * JT for q in range(Q)]
                for h in range(CH):
                    for q in range(Q):
                        gjs = slice(gj0s[q], gj0s[q] + JT)
                        nc.tensor.matmul(pss[q], bc.xh16[h][:, ibs],
                                         bc.xh16[h][:, gjs],
                                         start=(h == 0), stop=False)
                for stat, mov in ((bc.h8, bc.l8), (bc.l8, bc.h8)):
                    for q in range(Q):
                        gjs = slice(gj0s[q], gj0s[q] + JT)
                        nc.tensor.matmul(pss[q], stat[:, :, ibs],
                                         mov[:, :, gjs], start=False,
                                         stop=False, perf_mode=DR)
                for q in range(Q):
                    gjs = slice(gj0s[q], gj0s[q] + JT)
                    has_diag = gj0s[q] <= ib * P < gj0s[q] + JT
                    nc.tensor.matmul(pss[q], bc.ones2, bc.nsq[:, gjs],
                                     start=False, stop=not has_diag)
                for q in range(Q):
                    if gj0s[q] <= ib * P < gj0s[q] + JT:
                        ds = slice(ib * P - gj0s[q], ib * P - gj0s[q] + P)
                        nc.tensor.matmul(pss[q][:, ds], bc.ident16,
                                         bc.identbig, start=False, stop=True)
                for q in range(Q):
                    cs = slice(gj0s[q], gj0s[q] + JT)
                    nc.scalar.activation(nds[:, cs], pss[q], func=AF.Copy)
            # full-row top-8 scan on the evicted SBUF copy; self killed,
            # so ranks 0,1 are nn1, nn2 with exact indices from max_index
            m8a = mg.tile([P, 8], F32, tag="m8a", name=f"m8a_{b}_{ib}")
            i8a = mg.tile([P, 8], U32, tag="i8a", name=f"i8a_{b}_{ib}")
            nc.vector.max(out=m8a, in_=nds)
            nc.vector.max_index(out=i8a, in_max=m8a, in_values=nds)
            nc.gpsimd.tensor_copy(bc.idx12u[:, 2 * ib:2 * ib + 2],
                                  i8a[:, 0:2])
            # conv row gathers + g1+g2 sum
            g1 = mg.tile([P, C], F16, tag="g1", name=f"g1_{b}_{ib}")
            g2 = mg.tile([P, C], F16, tag="g2", name=f"g2_{b}_{ib}")
            nc.gpsimd.indirect_dma_start(
                out=g1, out_offset=None, in_=bc.ykt_d[0][:, :],
                in_offset=IndirectOffsetOnAxis(
                    ap=bc.idx12u[:, 2 * ib:2 * ib + 1], axis=0))
            nc.gpsimd.indirect_dma_start(
                out=g2, out_offset=None, in_=bc.ykt_d[1][:, :],
                in_offset=IndirectOffsetOnAxis(
                    ap=bc.idx12u[:, 2 * ib + 1:2 * ib + 2], axis=0))
            nc.gpsimd.tensor_tensor(out=bc.g12[:, ib * C:(ib + 1) * C],
                                    in0=g1, in1=g2, op=ALU.add)
            # conv output chunks interleave into the phase-A PE stream
            if ib >= 5 and (ib - 5) % (JT // P) == 0:
                _conv_chunk(nc, tc, bc, (ib - 5) // (JT // P))
        _conv_chunk(nc, tc, bc, NB // (JT // P) - 1)


def _conv_chunk(nc, tc, bc, ncn):
    """Y0 + transposed gather accumulate + biased eviction + out DMA
    for one 512-wide output chunk."""
    b = bc.b
    nsl = slice(ncn * JT, (ncn + 1) * JT)
    for oh in range(CH):
        ohs = slice(oh * P, (oh + 1) * P)
        pso = bc.pmisc.tile([P, JT], F32, tag="pm",
                            name=f"pso_{b}_{ncn}_{oh}")
        for h in range(CH):
            ws = slice(h * C + oh * P, h * C + (oh + 1) * P)
            nc.tensor.matmul(pso, bc.wt_sb[:, ws], bc.xh16[h][:, nsl],
                             start=(h == 0), stop=False)
        for nb4 in range(JT // P):
            ib = ncn * (JT // P) + nb4
            bs = slice(nb4 * P, (nb4 + 1) * P)
            gsl = slice(ib * C + oh * P, ib * C + (oh + 1) * P)
            nc.tensor.matmul(pso[:, bs], bc.g12[:, gsl], bc.ident16,
                             start=False, stop=True)
        osb = bc.mg.tile([P, JT], F32, tag="osb", name=f"osb_{b}_{ncn}_{oh}")
        nc.scalar.activation(osb, pso, func=AF.Identity,
                             bias=bc.biasc[oh][:, 0:1])
        nc.sync.dma_start(out=bc.out_t[b, ohs, nsl], in_=osb)


def build():
    nc = bacc.Bacc(None, target_bir_lowering=False)
    x_in = nc.dram_tensor("x", [BPC, C, N], F32, kind="ExternalInput")
    wt_in = nc.dram_tensor("wt", [K, C, C], F16, kind="ExternalInput")
    bias_in = nc.dram_tensor("bias", [C, 1], F32, kind="ExternalInput")
    out_t = nc.dram_tensor("out", [BPC, C, N], F32, kind="ExternalOutput")

    with TileContext(nc) as tc, contextlib.ExitStack() as es:
        constp = es.enter_context(tc.tile_pool(name="const", bufs=1))
        ident16 = constp.tile([P, P], F16)
        identbig = constp.tile([P, P], F16)
        ones_col = constp.tile([P, 1], F32)
        ones2 = constp.tile([2, P], F16)
        wt_sb = constp.tile([P, K * CH * C], F16)
        biasc = [constp.tile([P, 1], F32, tag=f"bc{oh}", name=f"bc{oh}")
                 for oh in range(CH)]
        make_identity(nc, ident16)
        nc.scalar.activation(identbig, ident16, func=AF.Copy, scale=-BIG)
        nc.vector.memset(ones_col, 1.0)
        nc.vector.memset(ones2, 1.0)
        for oh in range(CH):
            nc.sync.dma_start(out=biasc[oh],
                              in_=bias_in[oh * P:(oh + 1) * P, :])
        for h in range(CH):
            nc.sync.dma_start(out=wt_sb[:, h * C:(h + 1) * C],
                              in_=wt_in[0, h * P:(h + 1) * P, :])
            for k in (1, 2):
                ws = slice(W12_OFF + h * 2 * C + (k - 1) * C,
                           W12_OFF + h * 2 * C + k * C)
                nc.sync.dma_start(out=wt_sb[:, ws],
                                  in_=wt_in[k, h * P:(h + 1) * P, :])

        xb = es.enter_context(tc.tile_pool(name="xb", bufs=1))
        big = es.enter_context(tc.tile_pool(name="big", bufs=2))
        mg = es.enter_context(tc.tile_pool(name="mg", bufs=3))
        scratch = es.enter_context(tc.tile_pool(name="scratch", bufs=1))
        pmisc = es.enter_context(
            tc.tile_pool(name="pmisc", bufs=4, space="PSUM"))
        ydr = es.enter_context(tc.tile_pool(name="ydr", bufs=1, space="DRAM"))
        bcs = []
        for b in range(BPC):
            bc = BatchCtx()
            bc.b, bc.mg, bc.scratch, bc.xx = b, mg, scratch, []
            bc.big = big
            bc.pmisc, bc.out_t = pmisc, out_t
            bc.x_in, bc.wt_sb, bc.biasc = x_in, wt_sb, biasc
            bc.ident16, bc.identbig = ident16, identbig
            bc.ones_col, bc.ones2 = ones_col, ones2
            bc.xh16 = [xb.tile([P, N], F16, tag=f"xh{h}_{b}",
                               name=f"xh_{b}_{h}") for h in range(CH)]
            bc.h8 = xb.tile([P, CH, N], F8E4, tag=f"h8_{b}", name=f"h8_{b}")
            bc.l8 = xb.tile([P, CH, N], F8E5, tag=f"l8_{b}", name=f"l8_{b}")
            bc.nsq = xb.tile([2, N], F16, tag=f"nsq_{b}", name=f"nsq_{b}")
            bc.idx12u = xb.tile([P, NB * 2], U32, tag=f"i12u_{b}",
                                name=f"i12u_{b}")
            bc.g12 = xb.tile([P, NB * C], F16, tag=f"g12_{b}",
                             name=f"g12_{b}")
            bc.ykt_d = [ydr.tile([N, C], F16, tag=f"y{k}t_{b}",
                                 name=f"y{k}t_{b}") for k in (1, 2)]
            bcs.append(bc)

        for b in range(BPC):
            _prep_data(nc, tc, bcs[b])
            _prep_sq(nc, tc, bcs[b])
        # dummy matmuls keep the PE HAM monitor busy (warm 2.4 GHz)
        warm = pmisc.tile([P, P], F32, tag="pm", name="warm")
        for _ in range(96):
            nc.tensor.matmul(warm, ident16, ident16, start=True, stop=True)
        for b in range(BPC):
            _ykt(nc, tc, bcs[b])
        for b in range(BPC):
            _phase_a(nc, tc, bcs[b])
    nc.compile()
    return nc


_NC = None


def _get_nc():
    global _NC
    if _NC is None:
        _NC = build()
    return _NC


def make_in_maps(x, W, b):
    x = np.ascontiguousarray(x, dtype=np.float32)
    wt = np.ascontiguousarray(np.transpose(W, (2, 1, 0))).astype(np.float16)
    bias = np.ascontiguousarray(b, dtype=np.float32).reshape(C, 1)
    return [
        {"x": np.ascontiguousarray(x[i * BPC:(i + 1) * BPC]),
         "wt": wt, "bias": bias}
        for i in range(NCORES)
    ]


def kernel(x, W, b):
    nc = _get_nc()
    in_maps = make_in_maps(x, W, b)
    res = run_bass_kernel_spmd(nc, in_maps, core_ids=list(range(NCORES))).results
    return np.concatenate([r["out"] for r in res], axis=0)
